# revision 3
# baseline (speedup 1.0000x reference)
"""Trainium2 Bass kernel for nn_DecoderRNN (LSTM image-caption decoder).

Model: feat = features @ W_proj + b_proj;  x = [feat, emb[captions[:, :-1]]]
       LSTM over T=32 steps (batch 64, hidden 512), logits = hs @ W_fc + b_fc.

Distribution across 8 NeuronCores (no collectives):
  - The sequential LSTM recurrence is latency-bound (M=64), so it is
    REPLICATED on every core; each core therefore holds the full hidden
    trajectory locally.
  - The dominant FC layer (512x10000) and the 82MB output are SHARDED by
    vocab: core k computes logits[:, :, k*1250:(k+1)*1250].

Per-core kernel layout (everything transposed: partition = feature dim,
free = (time, batch)), all matmuls bf16 with fp32 PSUM accumulation:
  phase 1: proj   featT_E[m] = W_proj[:,m]^T @ features^T         (16 MM x 4)
  phase 2: gather emb rows via indirect DMA, DMA-transpose into X^T
  phase 3: XW^T   = Wx^T @ X^T + b (all timesteps, batched)       (256 MM)
  phase 4: recurrence; per step: 2 identity-MMs preload xw into PSUM,
           64 U-matmuls accumulate, ACT sigmoid/tanh, DVE gate math.
           h_t written (bf16) straight into the hs^T buffer.
  phase 5: FC per 8-step row chunk: logitsT = W_fc^T @ hs^T + b_fc,
           interleaved with the recurrence, streamed to DRAM.
"""

import numpy as np
import ml_dtypes

import concourse.bass as bass
import concourse.mybir as mybir
import concourse.tile as tile
from concourse import bacc
from concourse import bass_utils
from concourse.bass_interp import get_hw_module
from concourse.masks import make_identity

BF16 = mybir.dt.bfloat16
F32 = mybir.dt.float32
I32 = mybir.dt.int32
AF = mybir.ActivationFunctionType

B, T, E, H, V, IN = 64, 32, 512, 512, 10000, 2048
N_CORES = 8
VS = V // N_CORES          # 1250 vocab rows per core
VSP = 1280                 # padded to 10 x 128
G4 = 4 * H                 # 2048 gate columns, order (i, f, c, o)
R = B * T                  # 2048 rows, time-major: r = t*64 + b
KE = E // 128              # 4 contraction chunks over E/H
KIN = IN // 128            # 16 chunks over INPUT
MG = G4 // 128             # 16 gate chunks
NRN = 4                    # row chunks of 512 (8 timesteps each)
TPR = T // NRN             # 8 timesteps per row chunk
NVM = VSP // 128           # 10 vocab chunks per core


def build_program():
    nc = bacc.Bacc("TRN2", target_bir_lowering=False, debug=False)

    featT_d = nc.dram_tensor("featT", [IN, B], BF16, kind="ExternalInput").ap()
    wp_d = nc.dram_tensor("wp", [IN, E], BF16, kind="ExternalInput").ap()
    bp_d = nc.dram_tensor("bp", [128, KE], F32, kind="ExternalInput").ap()
    emb_d = nc.dram_tensor("emb", [V, E], BF16, kind="ExternalInput").ap()
    tok_d = nc.dram_tensor("tok", [128, 16], I32, kind="ExternalInput").ap()
    wx_d = nc.dram_tensor("wx", [E, G4], BF16, kind="ExternalInput").ap()
    u_d = nc.dram_tensor("u", [H, G4], BF16, kind="ExternalInput").ap()
    bc_d = nc.dram_tensor("bc", [128, MG], F32, kind="ExternalInput").ap()
    wfc_d = nc.dram_tensor("wfc", [H, VSP], BF16, kind="ExternalInput").ap()
    bfc_d = nc.dram_tensor("bfc", [128, NVM], F32, kind="ExternalInput").ap()
    logT_d = nc.dram_tensor("logT", [VSP, R], F32, kind="ExternalOutput").ap()

    with tile.TileContext(nc) as tc:
        with (
            tc.tile_pool(name="const", bufs=1) as const,
            tc.tile_pool(name="xt", bufs=1) as xtp,
            tc.tile_pool(name="xw", bufs=2) as xwp,
            tc.tile_pool(name="gath", bufs=3) as gathp,
            tc.tile_pool(name="hs", bufs=2) as hsp,
            tc.tile_pool(name="ct", bufs=2) as ctp,
            tc.tile_pool(name="elem", bufs=2) as elemp,
            tc.tile_pool(name="lg", bufs=3) as lgp,
            tc.tile_pool(name="gps", bufs=2, space="PSUM") as gpsp,
            tc.tile_pool(name="mmps", bufs=2, space="PSUM") as mmpsp,
        ):
            # ---- constants / weights to SBUF ----
            wx_sb = const.tile([128, KE, G4], BF16)
            nc.sync.dma_start(out=wx_sb, in_=wx_d.rearrange("(k p) j -> p k j", p=128))
            u_sb = const.tile([128, KE, G4], BF16)
            nc.sync.dma_start(out=u_sb, in_=u_d.rearrange("(k p) j -> p k j", p=128))
            wp_sb = const.tile([128, KIN, E], BF16)
            nc.sync.dma_start(out=wp_sb, in_=wp_d.rearrange("(k p) e -> p k e", p=128))
            wfc_sb = const.tile([128, KE, VSP], BF16)
            nc.sync.dma_start(out=wfc_sb, in_=wfc_d.rearrange("(k p) v -> p k v", p=128))
            featT_sb = const.tile([128, KIN, B], BF16)
            nc.sync.dma_start(out=featT_sb, in_=featT_d.rearrange("(k p) b -> p k b", p=128))
            bp_sb = const.tile([128, KE], F32)
            nc.sync.dma_start(out=bp_sb, in_=bp_d)
            bc_sb = const.tile([128, MG], F32)
            nc.sync.dma_start(out=bc_sb, in_=bc_d)
            bfc_sb = const.tile([128, NVM], F32)
            nc.sync.dma_start(out=bfc_sb, in_=bfc_d)
            tok_sb = const.tile([128, 16], I32)
            nc.sync.dma_start(out=tok_sb, in_=tok_d)
            ident = const.tile([128, 128], BF16)
            make_identity(nc, ident)

            # X^T: [E-within-chunk, E-chunk, row] rows time-major
            xT_sb = xtp.tile([128, KE, R], BF16)

            # ---- phase 1: feature projection -> x^T rows 0..63 ----
            for m in range(KE):
                ps = mmpsp.tile([128, 512], F32, tag="mmps")
                for k in range(KIN):
                    nc.tensor.matmul(
                        ps[:, :B],
                        lhsT=wp_sb[:, k, m * 128:(m + 1) * 128],
                        rhs=featT_sb[:, k, :],
                        start=(k == 0),
                        stop=(k == KIN - 1),
                    )
                nc.scalar.activation(
                    xT_sb[:, m, 0:B], ps[:, :B], AF.Identity,
                    bias=bp_sb[:, m:m + 1],
                )

            # ---- phase 2: embedding gather + transpose into X^T ----
            for g in range(16):
                gt = gathp.tile([128, E], BF16, tag="gath")
                nc.gpsimd.indirect_dma_start(
                    out=gt,
                    out_offset=None,
                    in_=emb_d,
                    in_offset=bass.IndirectOffsetOnAxis(ap=tok_sb[:, g:g + 1], axis=0),
                )
                w = min(128, R - B - g * 128)   # last tile: 64 valid tokens
                for e in range(KE):
                    nc.scalar.dma_start_transpose(
                        out=xT_sb[:, e, B + g * 128: B + g * 128 + w],
                        in_=gt[:w, e * 128:(e + 1) * 128],
                    )

            # ---- phase 3: XW^T (+ gate biases), row-chunk major ----
            xw_tiles = []
            for rn in range(NRN):
                xw_t = xwp.tile([128, MG, 512], BF16, tag="xw")
                xw_tiles.append(xw_t)
                for m in range(MG):
                    ps = mmpsp.tile([128, 512], F32, tag="mmps")
                    for k in range(KE):
                        nc.tensor.matmul(
                            ps,
                            lhsT=wx_sb[:, k, m * 128:(m + 1) * 128],
                            rhs=xT_sb[:, k, rn * 512:(rn + 1) * 512],
                            start=(k == 0),
                            stop=(k == KE - 1),
                        )
                    nc.scalar.activation(
                        xw_t[:, m, :], ps, AF.Identity, bias=bc_sb[:, m:m + 1],
                    )

            # ---- phases 4+5: recurrence with interleaved FC ----
            cT_prev = None
            hs_cur = None
            for t in range(T):
                rn, toff = divmod(t, TPR)
                if toff == 0:
                    hs_cur = hsp.tile([128, TPR, KE, B], BF16, tag="hs")
                xw_t = xw_tiles[rn]
                cT_new = ctp.tile([128, KE, B], F32, tag="ct")

                if t == 0:
                    # h = 0: gates come straight from xw
                    if_sb = elemp.tile([128, 512], F32, tag="if")
                    nc.scalar.activation(
                        if_sb.rearrange("p (m b) -> p m b", m=8),
                        xw_t[:, 0:8, 0:B], AF.Sigmoid)
                    g_sb = elemp.tile([128, 256], F32, tag="g")
                    nc.scalar.activation(
                        g_sb.rearrange("p (m b) -> p m b", m=4),
                        xw_t[:, 8:12, 0:B], AF.Tanh)
                    o_sb = elemp.tile([128, 256], F32, tag="o")
                    nc.scalar.activation(
                        o_sb.rearrange("p (m b) -> p m b", m=4),
                        xw_t[:, 12:16, 0:B], AF.Sigmoid)
                    # c = i*g
                    nc.vector.tensor_mul(
                        cT_new.rearrange("p k b -> p (k b)"),
                        if_sb[:, 0:256], g_sb)
                else:
                    gp = gpsp.tile([128, 1024], F32, tag="gp")
                    # preload xw into PSUM (sets has_written for accumulation)
                    nc.tensor.matmul(
                        gp[:, 0:512].rearrange("p (m b) -> p m b", m=8),
                        lhsT=ident, rhs=xw_t[:, 0:8, toff * B:(toff + 1) * B],
                        start=True, stop=False, skip_group_check=True)
                    nc.tensor.matmul(
                        gp[:, 512:1024].rearrange("p (m b) -> p m b", m=8),
                        lhsT=ident, rhs=xw_t[:, 8:16, toff * B:(toff + 1) * B],
                        start=True, stop=False, skip_group_check=True)
                    # h @ U accumulation
                    hprev_rn, hprev_off = divmod(t - 1, TPR)
                    hs_prev = hs_cur if hprev_rn == rn else hs_prevchunk
                    for m in range(MG):
                        for k in range(KE):
                            nc.tensor.matmul(
                                gp[:, m * B:(m + 1) * B],
                                lhsT=u_sb[:, k, m * 128:(m + 1) * 128],
                                rhs=hs_prev[:, hprev_off, k, :],
                                start=False, stop=(k == KE - 1),
                                skip_group_check=True)
                    if_sb = elemp.tile([128, 512], F32, tag="if")
                    nc.scalar.activation(if_sb, gp[:, 0:512], AF.Sigmoid)
                    g_sb = elemp.tile([128, 256], F32, tag="g")
                    nc.scalar.activation(g_sb, gp[:, 512:768], AF.Tanh)
                    o_sb = elemp.tile([128, 256], F32, tag="o")
                    nc.scalar.activation(o_sb, gp[:, 768:1024], AF.Sigmoid)
                    fc_t = elemp.tile([128, 256], F32, tag="fc")
                    nc.vector.tensor_mul(
                        fc_t, if_sb[:, 256:512],
                        cT_prev.rearrange("p k b -> p (k b)"))
                    ig_t = elemp.tile([128, 256], F32, tag="ig")
                    nc.vector.tensor_mul(ig_t, if_sb[:, 0:256], g_sb)
                    nc.vector.tensor_add(
                        cT_new.rearrange("p k b -> p (k b)"), fc_t, ig_t)

                tanhc = elemp.tile([128, 256], F32, tag="tanhc")
                nc.scalar.activation(
                    tanhc, cT_new.rearrange("p k b -> p (k b)"), AF.Tanh)
                # h_t = o * tanh(c) -> straight into hs^T (bf16)
                nc.vector.tensor_mul(
                    hs_cur[:, toff, :, :],
                    o_sb.rearrange("p (k b) -> p k b", k=KE),
                    tanhc.rearrange("p (k b) -> p k b", k=KE))
                cT_prev = cT_new

                if toff == TPR - 1:
                    hs_prevchunk = hs_cur
                    # FC for this row chunk
                    for vm in range(NVM):
                        ps = mmpsp.tile([128, 512], F32, tag="mmps")
                        for k in range(KE):
                            nc.tensor.matmul(
                                ps.rearrange("p (a b) -> p a b", a=TPR),
                                lhsT=wfc_sb[:, k, vm * 128:(vm + 1) * 128],
                                rhs=hs_cur[:, :, k, :],
                                start=(k == 0),
                                stop=(k == KE - 1),
                            )
                        lg = lgp.tile([128, 512], F32, tag="lg")
                        nc.scalar.activation(
                            lg, ps, AF.Identity, bias=bfc_sb[:, vm:vm + 1])
                        nc.sync.dma_start(
                            out=logT_d[vm * 128:(vm + 1) * 128,
                                       rn * 512:(rn + 1) * 512],
                            in_=lg)

    nc.compile()
    return nc


def prep_inputs(inputs):
    """Host-side sharding/layout prep. Returns in_maps for 8 cores."""
    bf = lambda a: np.ascontiguousarray(np.asarray(a, np.float32)).astype(
        ml_dtypes.bfloat16)
    f32 = lambda a: np.ascontiguousarray(np.asarray(a, np.float32))

    features = f32(inputs["features"])
    captions = np.asarray(inputs["captions"]).astype(np.int32)
    wx = np.concatenate([f32(inputs["W_i"]), f32(inputs["W_f"]),
                         f32(inputs["W_c"]), f32(inputs["W_o"])], axis=1)
    u = np.concatenate([f32(inputs["U_i"]), f32(inputs["U_f"]),
                        f32(inputs["U_c"]), f32(inputs["U_o"])], axis=1)
    bc = np.concatenate([f32(inputs["b_i"]), f32(inputs["b_f"]),
                         f32(inputs["b_c"]), f32(inputs["b_o"])])
    wfc = f32(inputs["W_fc"])
    bfc = f32(inputs["b_fc"])

    # time-major token table for x rows 64..2047, packed [128, 16]
    tok = captions[:, :T - 1].T.reshape(-1)          # [(t-1)*64 + b], 1984
    tok = np.concatenate([tok, np.zeros(2048 - tok.size, np.int32)])
    tok2d = np.ascontiguousarray(tok.reshape(16, 128).T).astype(np.int32)

    common = {
        "featT": bf(features.T),
        "wp": bf(inputs["W_proj"]),
        "bp": f32(inputs["b_proj"]).reshape(KE, 128).T.copy(),
        "emb": bf(inputs["emb"]),
        "tok": tok2d,
        "wx": bf(wx),
        "u": bf(u),
        "bc": bc.reshape(MG, 128).T.copy(),
    }
    in_maps = []
    for c in range(N_CORES):
        wfc_k = np.zeros((H, VSP), np.float32)
        wfc_k[:, :VS] = wfc[:, c * VS:(c + 1) * VS]
        bfc_k = np.zeros((VSP,), np.float32)
        bfc_k[:VS] = bfc[c * VS:(c + 1) * VS]
        in_maps.append({
            **common,
            "wfc": bf(wfc_k),
            "bfc": bfc_k.reshape(NVM, 128).T.copy(),
        })
    return in_maps


def assemble(results):
    """results: list of 8 dicts with 'logT' [VSP, R] -> [B, T, V] f32."""
    full = np.concatenate([np.asarray(r["logT"])[:VS] for r in results], axis=0)
    return np.ascontiguousarray(
        full.reshape(V, T, B).transpose(2, 1, 0)).astype(np.float32)


_CACHED_NC = None


def _get_nc():
    global _CACHED_NC
    if _CACHED_NC is None:
        _CACHED_NC = build_program()
        _CACHED_NC.m = get_hw_module(_CACHED_NC.m)
    return _CACHED_NC


def run_on_hw(inputs, trace=False):
    nc = _get_nc()
    in_maps = prep_inputs(inputs)
    res = bass_utils.run_bass_kernel_spmd(
        nc, in_maps, core_ids=list(range(N_CORES)), trace=trace)
    return assemble(res.results), res.exec_time_ns


def kernel(**inputs):
    out, _ = run_on_hw(inputs, trace=False)
    return out


# revision 6
# speedup vs baseline: 1.2441x; 1.2441x over previous
"""Trainium2 Bass kernel for nn_DecoderRNN (LSTM image-caption decoder).

Model: feat = features @ W_proj + b_proj;  x = [feat, emb[captions[:, :-1]]]
       LSTM over T=32 steps (batch 64, hidden 512), logits = hs @ W_fc + b_fc.

Distribution across 8 NeuronCores (no collectives):
  - The sequential LSTM recurrence is latency-bound (M=64), so it is
    REPLICATED on every core; each core therefore holds the full hidden
    trajectory locally.
  - The dominant FC layer (512x10000) and the 82MB output are SHARDED by
    vocab: core k computes logits[:, :, k*1250:(k+1)*1250].

Per-core kernel layout (everything transposed: partition = feature dim,
free = (time, batch)), all matmuls bf16 with fp32 PSUM accumulation:
  - proj: featT_E[m] = W_proj[:,m]^T @ features^T
  - gather emb rows via indirect DMA, DMA-transpose into X^T (row chunks)
  - XW^T = Wx^T @ X^T + b, one row chunk (8 steps) at a time
  - recurrence per step: 2 identity-matmuls preload xw into PSUM (sets
    has_written), 64 U-matmuls accumulate, gate order c,f,i,o so the
    ACT/DVE chain starts early; h_t written bf16 straight into hs^T
  - FC logitsT = W_fc^T @ hs^T + b_fc per row chunk, streamed to DRAM
  XW chunks for row-chunk rn+1 and FC chunks for rn-1 are emitted BETWEEN
  recurrence steps: the PE executes its queue in order, so this keeps the
  tensor engine dense (HAM stays warm) without delaying the next step.
"""

import numpy as np
import ml_dtypes

import concourse.bass as bass
import concourse.mybir as mybir
import concourse.tile as tile
from concourse import bacc
from concourse import bass_utils
from concourse.bass_interp import get_hw_module
from concourse.masks import make_identity

BF16 = mybir.dt.bfloat16
F32 = mybir.dt.float32
I32 = mybir.dt.int32
AF = mybir.ActivationFunctionType

B, T, E, H, V, IN = 64, 32, 512, 512, 10000, 2048
N_CORES = 8
VS = V // N_CORES          # 1250 vocab rows per core
VSP = 1280                 # padded to 10 x 128
G4 = 4 * H                 # 2048 gate columns, order (i, f, c, o)
R = B * T                  # 2048 rows, time-major: r = t*64 + b
KE = E // 128              # 4 contraction chunks over E/H
KIN = IN // 128            # 16 chunks over INPUT
MG = G4 // 128             # 16 gate chunks
NRN = 4                    # row chunks of 512 (8 timesteps each)
TPR = T // NRN             # 8 timesteps per row chunk
NVM = VSP // 128           # 10 vocab chunks per core

# U-matmul emission order: c first (tanh can start), then f, i, o
M_ORDER = [8, 9, 10, 11, 4, 5, 6, 7, 0, 1, 2, 3, 12, 13, 14, 15]


def build_program():
    nc = bacc.Bacc("TRN2", target_bir_lowering=False, debug=False)

    featT_d = nc.dram_tensor("featT", [IN, B], BF16, kind="ExternalInput").ap()
    wp_d = nc.dram_tensor("wp", [IN, E], BF16, kind="ExternalInput").ap()
    bp_d = nc.dram_tensor("bp", [128, KE], F32, kind="ExternalInput").ap()
    emb_d = nc.dram_tensor("emb", [V, E], BF16, kind="ExternalInput").ap()
    tok_d = nc.dram_tensor("tok", [128, 16], I32, kind="ExternalInput").ap()
    wx_d = nc.dram_tensor("wx", [E, G4], BF16, kind="ExternalInput").ap()
    u_d = nc.dram_tensor("u", [H, G4], BF16, kind="ExternalInput").ap()
    bc_d = nc.dram_tensor("bc", [128, MG], F32, kind="ExternalInput").ap()
    wfc_d = nc.dram_tensor("wfc", [H, VSP], BF16, kind="ExternalInput").ap()
    bfc_d = nc.dram_tensor("bfc", [128, NVM], F32, kind="ExternalInput").ap()
    logT_d = nc.dram_tensor("logT", [VSP, R], F32, kind="ExternalOutput").ap()

    with tile.TileContext(nc) as tc:
        with (
            tc.tile_pool(name="const", bufs=1) as const,
            tc.tile_pool(name="xt", bufs=1) as xtp,
            tc.tile_pool(name="xw", bufs=2) as xwp,
            tc.tile_pool(name="gath", bufs=4) as gathp,
            tc.tile_pool(name="hs", bufs=2) as hsp,
            tc.tile_pool(name="ct", bufs=2) as ctp,
            tc.tile_pool(name="elem", bufs=2) as elemp,
            tc.tile_pool(name="lg", bufs=3) as lgp,
            tc.tile_pool(name="gps", bufs=2, space="PSUM") as gpsp,
            tc.tile_pool(name="mmps", bufs=2, space="PSUM") as mmpsp,
        ):
            # ---- weights/constants to SBUF (sync queue, early-need first) ---
            tok_sb = const.tile([128, 16], I32)
            nc.sync.dma_start(out=tok_sb, in_=tok_d)
            wp_sb = const.tile([128, KIN, E], BF16)
            nc.sync.dma_start(out=wp_sb, in_=wp_d.rearrange("(k p) e -> p k e", p=128))
            featT_sb = const.tile([128, KIN, B], BF16)
            nc.sync.dma_start(out=featT_sb, in_=featT_d.rearrange("(k p) b -> p k b", p=128))
            wx_sb = const.tile([128, KE, G4], BF16)
            nc.sync.dma_start(out=wx_sb, in_=wx_d.rearrange("(k p) j -> p k j", p=128))
            u_sb = const.tile([128, KE, G4], BF16)
            nc.sync.dma_start(out=u_sb, in_=u_d.rearrange("(k p) j -> p k j", p=128))
            wfc_sb = const.tile([128, KE, VSP], BF16)
            nc.sync.dma_start(out=wfc_sb, in_=wfc_d.rearrange("(k p) v -> p k v", p=128))
            bp_sb = const.tile([128, KE], F32)
            nc.sync.dma_start(out=bp_sb, in_=bp_d)
            bc_sb = const.tile([128, MG], F32)
            nc.sync.dma_start(out=bc_sb, in_=bc_d)
            bfc_sb = const.tile([128, NVM], F32)
            nc.sync.dma_start(out=bfc_sb, in_=bfc_d)
            ident = const.tile([128, 128], BF16)
            make_identity(nc, ident)

            # X^T row-chunk tiles: [E-within-chunk, E-chunk, row-in-chunk]
            xT = [xtp.tile([128, KE, 512], BF16, tag=f"xt{rn}", name=f"xt{rn}")
                  for rn in range(NRN)]

            # ---- embedding gather + DMA-transpose into X^T ----
            for g in range(16):
                gt = gathp.tile([128, E], BF16, tag="gath", name="gt")
                nc.gpsimd.indirect_dma_start(
                    out=gt,
                    out_offset=None,
                    in_=emb_d,
                    in_offset=bass.IndirectOffsetOnAxis(ap=tok_sb[:, g:g + 1], axis=0),
                )
                # rows B + 128*g + (0..w) split across row-chunk boundaries
                r0 = B + g * 128
                w = min(128, R - r0)
                # early chunks (needed first) go on the scalar queue; the
                # sync queue is busy with weight loads at the start.
                qeng = nc.scalar if g < 4 else (nc.scalar if g % 2 else nc.sync)
                segs = []
                j = 0
                while j < w:
                    rn, off = divmod(r0 + j, 512)
                    seg = min(w - j, 512 - off)
                    segs.append((j, rn, off, seg))
                    j += seg
                for e in range(KE):
                    for (j, rn, off, seg) in segs:
                        qeng.dma_start_transpose(
                            out=xT[rn][:, e, off:off + seg],
                            in_=gt[j:j + seg, e * 128:(e + 1) * 128],
                        )

            # ---- feature projection -> x^T rows 0..63 (chunk 0) ----
            for m in range(KE):
                ps = mmpsp.tile([128, 512], F32, tag="mmps", name="ps")
                for k in range(KIN):
                    nc.tensor.matmul(
                        ps[:, :B],
                        lhsT=wp_sb[:, k, m * 128:(m + 1) * 128],
                        rhs=featT_sb[:, k, :],
                        start=(k == 0),
                        stop=(k == KIN - 1),
                    )
                nc.scalar.activation(
                    xT[0][:, m, 0:B], ps[:, :B], AF.Identity,
                    bias=bp_sb[:, m:m + 1],
                )

            xw_tiles = [None] * NRN

            def emit_xw_chunk(rn, m):
                if xw_tiles[rn] is None:
                    xw_tiles[rn] = xwp.tile([128, MG, 512], BF16, tag="xw", name=f"xw{rn}")
                xw_t = xw_tiles[rn]
                ps = mmpsp.tile([128, 512], F32, tag="mmps", name="ps")
                for k in range(KE):
                    nc.tensor.matmul(
                        ps,
                        lhsT=wx_sb[:, k, m * 128:(m + 1) * 128],
                        rhs=xT[rn][:, k, :],
                        start=(k == 0),
                        stop=(k == KE - 1),
                    )
                nc.scalar.activation(
                    xw_t[:, m, :], ps, AF.Identity, bias=bc_sb[:, m:m + 1])

            def emit_fc_chunk(rn, vm, hs_tile):
                ps = mmpsp.tile([128, 512], F32, tag="mmps", name="ps")
                for k in range(KE):
                    nc.tensor.matmul(
                        ps.rearrange("p (a b) -> p a b", a=TPR),
                        lhsT=wfc_sb[:, k, vm * 128:(vm + 1) * 128],
                        rhs=hs_tile[:, :, k, :],
                        start=(k == 0),
                        stop=(k == KE - 1),
                    )
                lg = lgp.tile([128, 512], F32, tag="lg", name="lg")
                nc.scalar.activation(lg, ps, AF.Identity, bias=bfc_sb[:, vm:vm + 1])
                nc.sync.dma_start(
                    out=logT_d[vm * 128:(vm + 1) * 128, rn * 512:(rn + 1) * 512],
                    in_=lg)

            # XW for row chunk 0 must precede the loop
            for m in range(MG):
                emit_xw_chunk(0, m)

            # ---- recurrence with interleaved XW / FC ----
            cT_prev = None
            hs_cur = None
            hs_done = None           # completed hs tile of the previous chunk
            fc_queue = []            # (rn, vm, hs_tile) chunks awaiting emission
            for t in range(T):
                rn, toff = divmod(t, TPR)
                if toff == 0:
                    hs_cur = hsp.tile([128, TPR, KE, B], BF16, tag="hs", name="hs")
                xw_t = xw_tiles[rn]
                cT_new = ctp.tile([128, KE, B], F32, tag="ct", name="ct")

                if t == 0:
                    if_sb = elemp.tile([128, 512], F32, tag="if")
                    nc.scalar.activation(
                        if_sb.rearrange("p (m b) -> p m b", m=8),
                        xw_t[:, 0:8, 0:B], AF.Sigmoid)
                    g_sb = elemp.tile([128, 256], F32, tag="g")
                    nc.scalar.activation(
                        g_sb.rearrange("p (m b) -> p m b", m=4),
                        xw_t[:, 8:12, 0:B], AF.Tanh)
                    o_sb = elemp.tile([128, 256], F32, tag="o")
                    nc.scalar.activation(
                        o_sb.rearrange("p (m b) -> p m b", m=4),
                        xw_t[:, 12:16, 0:B], AF.Sigmoid)
                    nc.vector.tensor_mul(
                        cT_new.rearrange("p k b -> p (k b)"),
                        if_sb[:, 0:256], g_sb)
                else:
                    hprev_rn, hprev_off = divmod(t - 1, TPR)
                    hs_prev = hs_cur if hprev_rn == rn else hs_done
                    gp = gpsp.tile([128, 1024], F32, tag="gp", name="gp")
                    # preload xw into PSUM (sets has_written for accumulation)
                    nc.tensor.matmul(
                        gp[:, 0:512].rearrange("p (m b) -> p m b", m=8),
                        lhsT=ident, rhs=xw_t[:, 0:8, toff * B:(toff + 1) * B],
                        start=True, stop=False, skip_group_check=True)
                    nc.tensor.matmul(
                        gp[:, 512:1024].rearrange("p (m b) -> p m b", m=8),
                        lhsT=ident, rhs=xw_t[:, 8:16, toff * B:(toff + 1) * B],
                        start=True, stop=False, skip_group_check=True)
                    for m in M_ORDER:
                        for k in range(KE):
                            nc.tensor.matmul(
                                gp[:, m * B:(m + 1) * B],
                                lhsT=u_sb[:, k, m * 128:(m + 1) * 128],
                                rhs=hs_prev[:, hprev_off, k, :],
                                start=False, stop=(k == KE - 1),
                                skip_group_check=True)
                    # gate slices (i,f,c,o layout): c first for early tanh
                    g_sb = elemp.tile([128, 256], F32, tag="g")
                    nc.scalar.activation(g_sb, gp[:, 512:768], AF.Tanh)
                    if_sb = elemp.tile([128, 512], F32, tag="if")
                    nc.scalar.activation(if_sb[:, 256:512], gp[:, 256:512], AF.Sigmoid)
                    nc.scalar.activation(if_sb[:, 0:256], gp[:, 0:256], AF.Sigmoid)
                    o_sb = elemp.tile([128, 256], F32, tag="o")
                    nc.scalar.activation(o_sb, gp[:, 768:1024], AF.Sigmoid)
                    fc_t = elemp.tile([128, 256], F32, tag="fc")
                    nc.vector.tensor_mul(
                        fc_t, if_sb[:, 256:512],
                        cT_prev.rearrange("p k b -> p (k b)"))
                    ig_t = elemp.tile([128, 256], F32, tag="ig")
                    nc.vector.tensor_mul(ig_t, if_sb[:, 0:256], g_sb)
                    nc.vector.tensor_add(
                        cT_new.rearrange("p k b -> p (k b)"), fc_t, ig_t)

                tanhc = elemp.tile([128, 256], F32, tag="tanhc")
                nc.scalar.activation(
                    tanhc, cT_new.rearrange("p k b -> p (k b)"), AF.Tanh)
                # h_t = o * tanh(c) -> straight into hs^T (bf16)
                nc.vector.tensor_mul(
                    hs_cur[:, toff, :, :],
                    o_sb.rearrange("p (k b) -> p k b", k=KE),
                    tanhc.rearrange("p (k b) -> p k b", k=KE))
                cT_prev = cT_new

                # fill PE stream: 2 XW chunks of the next row chunk,
                # then up to 2 pending FC chunks of the previous one
                if rn + 1 < NRN:
                    for m in (2 * toff, 2 * toff + 1):
                        emit_xw_chunk(rn + 1, m)
                for _ in range(2):
                    if fc_queue:
                        emit_fc_chunk(*fc_queue.pop(0))

                if toff == TPR - 1:
                    hs_done = hs_cur
                    fc_queue.extend((rn, vm, hs_cur) for vm in range(NVM))

            # drain remaining FC chunks (last row chunk)
            for args in fc_queue:
                emit_fc_chunk(*args)

    nc.compile()
    return nc


def prep_inputs(inputs):
    """Host-side sharding/layout prep. Returns in_maps for 8 cores."""
    bf = lambda a: np.ascontiguousarray(np.asarray(a, np.float32)).astype(
        ml_dtypes.bfloat16)
    f32 = lambda a: np.ascontiguousarray(np.asarray(a, np.float32))

    features = f32(inputs["features"])
    captions = np.asarray(inputs["captions"]).astype(np.int32)
    wx = np.concatenate([f32(inputs["W_i"]), f32(inputs["W_f"]),
                         f32(inputs["W_c"]), f32(inputs["W_o"])], axis=1)
    u = np.concatenate([f32(inputs["U_i"]), f32(inputs["U_f"]),
                        f32(inputs["U_c"]), f32(inputs["U_o"])], axis=1)
    bc = np.concatenate([f32(inputs["b_i"]), f32(inputs["b_f"]),
                         f32(inputs["b_c"]), f32(inputs["b_o"])])
    wfc = f32(inputs["W_fc"])
    bfc = f32(inputs["b_fc"])

    # time-major token table for x rows 64..2047, packed [128, 16]
    tok = captions[:, :T - 1].T.reshape(-1)          # [(t-1)*64 + b], 1984
    tok = np.concatenate([tok, np.zeros(2048 - tok.size, np.int32)])
    tok2d = np.ascontiguousarray(tok.reshape(16, 128).T).astype(np.int32)

    common = {
        "featT": bf(features.T),
        "wp": bf(inputs["W_proj"]),
        "bp": f32(inputs["b_proj"]).reshape(KE, 128).T.copy(),
        "emb": bf(inputs["emb"]),
        "tok": tok2d,
        "wx": bf(wx),
        "u": bf(u),
        "bc": bc.reshape(MG, 128).T.copy(),
    }
    in_maps = []
    for c in range(N_CORES):
        wfc_k = np.zeros((H, VSP), np.float32)
        wfc_k[:, :VS] = wfc[:, c * VS:(c + 1) * VS]
        bfc_k = np.zeros((VSP,), np.float32)
        bfc_k[:VS] = bfc[c * VS:(c + 1) * VS]
        in_maps.append({
            **common,
            "wfc": bf(wfc_k),
            "bfc": bfc_k.reshape(NVM, 128).T.copy(),
        })
    return in_maps


def assemble(results):
    """results: list of 8 dicts with 'logT' [VSP, R] -> [B, T, V] f32."""
    full = np.concatenate([np.asarray(r["logT"])[:VS] for r in results], axis=0)
    return np.ascontiguousarray(
        full.reshape(V, T, B).transpose(2, 1, 0)).astype(np.float32)


_CACHED_NC = None


def _get_nc():
    global _CACHED_NC
    if _CACHED_NC is None:
        _CACHED_NC = build_program()
        _CACHED_NC.m = get_hw_module(_CACHED_NC.m)
    return _CACHED_NC


def run_on_hw(inputs, trace=False):
    nc = _get_nc()
    in_maps = prep_inputs(inputs)
    res = bass_utils.run_bass_kernel_spmd(
        nc, in_maps, core_ids=list(range(N_CORES)), trace=trace)
    return assemble(res.results), res.exec_time_ns


def kernel(**inputs):
    out, _ = run_on_hw(inputs, trace=False)
    return out


# revision 8
# speedup vs baseline: 1.5425x; 1.2399x over previous
"""Trainium2 Bass kernel for nn_DecoderRNN (LSTM image-caption decoder).

Model: feat = features @ W_proj + b_proj;  x = [feat, emb[captions[:, :-1]]]
       LSTM over T=32 steps (batch 64, hidden 512), logits = hs @ W_fc + b_fc.

Distribution across 8 NeuronCores (no collectives):
  - The sequential LSTM recurrence is latency-bound (M=64), so it is
    REPLICATED on every core; each core therefore holds the full hidden
    trajectory locally.
  - The dominant FC layer (512x10000) and the 82MB output are SHARDED by
    vocab: core k computes logits[:, :, k*1250:(k+1)*1250].

Per-core kernel layout (everything transposed: partition = feature dim,
free = (time, batch)), all matmuls bf16 with fp32 PSUM accumulation:
  - weights arrive host-pretiled so every load is one contiguous DMA
  - proj: featT_E[m] = W_proj[:,m]^T @ features^T
  - embedding rows gathered by indirect DMA, transposed on the PE
    (128x128 transpose + copy) into X^T row-chunk tiles
  - XW^T = Wx^T @ X^T + b, one row chunk (8 steps) at a time
  - recurrence per step: 2 identity-matmuls preload xw into PSUM (sets
    has_written), 64 U-matmuls accumulate, gate order c,f,i,o so the
    ACT/DVE chain starts early; h_t written bf16 straight into hs^T
  - FC logitsT = W_fc^T @ hs^T + b_fc per row chunk, streamed to DRAM
  Transposes, XW chunks of row-chunk rn+1 and FC chunks of rn-1 are
  emitted BETWEEN recurrence steps: each engine executes its queue in
  order, so this fills the tensor engine's dependency-chain gaps (HAM
  stays warm) without delaying the next step.
"""

import numpy as np
import ml_dtypes

import concourse.bass as bass
import concourse.mybir as mybir
import concourse.tile as tile
from concourse import bacc
from concourse import bass_utils
from concourse.bass_interp import get_hw_module
from concourse.masks import make_identity

BF16 = mybir.dt.bfloat16
F32 = mybir.dt.float32
I32 = mybir.dt.int32
AF = mybir.ActivationFunctionType
ALU = mybir.AluOpType

B, T, E, H, V, IN = 64, 32, 512, 512, 10000, 2048
N_CORES = 8
VS = V // N_CORES          # 1250 vocab rows per core
VSP = 1280                 # padded to 10 x 128
G4 = 4 * H                 # 2048 gate columns, order (i, f, c, o)
R = B * T                  # 2048 rows, time-major: r = t*64 + b
KE = E // 128              # 4 contraction chunks over E/H
KIN = IN // 128            # 16 chunks over INPUT
MG = G4 // 128             # 16 gate chunks
NRN = 4                    # row chunks of 512 (8 timesteps each)
TPR = T // NRN             # 8 timesteps per row chunk
NVM = VSP // 128           # 10 vocab chunks per core

# U-matmul emission order: c first (tanh can start), then f, i, o
M_ORDER = [8, 9, 10, 11, 4, 5, 6, 7, 0, 1, 2, 3, 12, 13, 14, 15]


def _row_segments(g):
    """Token tile g holds x rows r0..r0+w; split by row-chunk boundary."""
    r0 = B + g * 128
    w = min(128, R - r0)
    segs = []
    j = 0
    while j < w:
        rn, off = divmod(r0 + j, 512)
        seg = min(w - j, 512 - off)
        segs.append((j, rn, off, seg))
        j += seg
    return segs


def build_program():
    nc = bacc.Bacc("TRN2", target_bir_lowering=False, debug=False)

    featT_d = nc.dram_tensor("featT", [128, KIN * B], BF16, kind="ExternalInput").ap()
    wp_d = nc.dram_tensor("wp", [128, KIN * E], BF16, kind="ExternalInput").ap()
    bp_d = nc.dram_tensor("bp", [128, KE], F32, kind="ExternalInput").ap()
    emb_d = nc.dram_tensor("emb", [V, E], BF16, kind="ExternalInput").ap()
    tok_d = nc.dram_tensor("tok", [128, 16], I32, kind="ExternalInput").ap()
    wx_d = nc.dram_tensor("wx", [128, KE * G4], BF16, kind="ExternalInput").ap()
    u_d = nc.dram_tensor("u", [128, KE * G4], BF16, kind="ExternalInput").ap()
    bc_d = nc.dram_tensor("bc", [128, MG], F32, kind="ExternalInput").ap()
    wfc_d = nc.dram_tensor("wfc", [128, KE * VSP], BF16, kind="ExternalInput").ap()
    bfc_d = nc.dram_tensor("bfc", [128, NVM], F32, kind="ExternalInput").ap()
    logT_d = nc.dram_tensor("logT", [VSP, R], F32, kind="ExternalOutput").ap()

    with tile.TileContext(nc) as tc:
        with (
            tc.tile_pool(name="const", bufs=1) as const,
            tc.tile_pool(name="xt", bufs=1) as xtp,
            tc.tile_pool(name="xw", bufs=2) as xwp,
            tc.tile_pool(name="gath", bufs=4) as gathp,
            tc.tile_pool(name="hs", bufs=2) as hsp,
            tc.tile_pool(name="ct", bufs=2) as ctp,
            tc.tile_pool(name="elem", bufs=2) as elemp,
            tc.tile_pool(name="lg", bufs=3) as lgp,
            tc.tile_pool(name="gps", bufs=2, space="PSUM") as gpsp,
            tc.tile_pool(name="mmps", bufs=2, space="PSUM") as mmpsp,
            tc.tile_pool(name="tps", bufs=2, space="PSUM") as tpsp,
        ):
            # ---- weights/constants to SBUF (contiguous, need-ordered) ----
            tok_sb = const.tile([128, 16], I32)
            nc.sync.dma_start(out=tok_sb, in_=tok_d)
            wp_sb = const.tile([128, KIN, E], BF16)
            nc.sync.dma_start(out=wp_sb.rearrange("p k e -> p (k e)"), in_=wp_d)
            featT_sb = const.tile([128, KIN, B], BF16)
            nc.sync.dma_start(out=featT_sb.rearrange("p k b -> p (k b)"), in_=featT_d)
            wx_sb = const.tile([128, KE, G4], BF16)
            nc.sync.dma_start(out=wx_sb.rearrange("p k j -> p (k j)"), in_=wx_d)
            u_sb = const.tile([128, KE, G4], BF16)
            nc.sync.dma_start(out=u_sb.rearrange("p k j -> p (k j)"), in_=u_d)
            wfc_sb = const.tile([128, KE, VSP], BF16)
            nc.sync.dma_start(out=wfc_sb.rearrange("p k v -> p (k v)"), in_=wfc_d)
            bp_sb = const.tile([128, KE], F32)
            nc.sync.dma_start(out=bp_sb, in_=bp_d)
            bc_sb = const.tile([128, MG], F32)
            nc.sync.dma_start(out=bc_sb, in_=bc_d)
            bfc_sb = const.tile([128, NVM], F32)
            nc.sync.dma_start(out=bfc_sb, in_=bfc_d)
            ident = const.tile([128, 128], BF16)
            make_identity(nc, ident)

            # X^T row-chunk tiles: [E-within-chunk, E-chunk, row-in-chunk]
            xT = [xtp.tile([128, KE, 512], BF16, tag=f"xt{rn}", name=f"xt{rn}")
                  for rn in range(NRN)]

            # ---- all embedding gathers up front (gpsimd queue) ----
            gts = []
            for g in range(16):
                gt = gathp.tile([128, E], BF16, tag="gath", name="gt")
                nc.gpsimd.indirect_dma_start(
                    out=gt,
                    out_offset=None,
                    in_=emb_d,
                    in_offset=bass.IndirectOffsetOnAxis(ap=tok_sb[:, g:g + 1], axis=0),
                )
                gts.append(gt)

            def emit_transpose(g):
                """PE-transpose token tile g into its X^T slots."""
                gt = gts[g]
                segs = _row_segments(g)
                for e in range(KE):
                    tp = tpsp.tile([128, 128], BF16, tag="tp", name="tp")
                    nc.tensor.transpose(tp, gt[:, e * 128:(e + 1) * 128], ident)
                    for (j, rn, off, seg) in segs:
                        nc.scalar.activation(
                            xT[rn][:, e, off:off + seg], tp[:, j:j + seg], AF.Copy)

            # ---- feature projection -> x^T rows 0..63 (chunk 0) ----
            for m in range(KE):
                ps = mmpsp.tile([128, 512], F32, tag="mmps", name="ps")
                for k in range(KIN):
                    nc.tensor.matmul(
                        ps[:, :B],
                        lhsT=wp_sb[:, k, m * 128:(m + 1) * 128],
                        rhs=featT_sb[:, k, :],
                        start=(k == 0),
                        stop=(k == KIN - 1),
                    )
                nc.scalar.activation(
                    xT[0][:, m, 0:B], ps[:, :B], AF.Identity,
                    bias=bp_sb[:, m:m + 1],
                )

            for g in range(4):
                emit_transpose(g)

            xw_tiles = [None] * NRN

            def emit_xw_chunk(rn, m):
                if xw_tiles[rn] is None:
                    xw_tiles[rn] = xwp.tile([128, MG, 512], BF16, tag="xw",
                                            name=f"xw{rn}")
                xw_t = xw_tiles[rn]
                ps = mmpsp.tile([128, 512], F32, tag="mmps", name="ps")
                for k in range(KE):
                    nc.tensor.matmul(
                        ps,
                        lhsT=wx_sb[:, k, m * 128:(m + 1) * 128],
                        rhs=xT[rn][:, k, :],
                        start=(k == 0),
                        stop=(k == KE - 1),
                    )
                # copy + gate bias on DVE (ACT is the busier engine in-loop)
                nc.vector.tensor_scalar(
                    out=xw_t[:, m, :], in0=ps, scalar1=bc_sb[:, m:m + 1],
                    scalar2=None, op0=ALU.add)

            def emit_fc_chunk(rn, vm, hs_tile):
                ps = mmpsp.tile([128, 512], F32, tag="mmps", name="ps")
                for k in range(KE):
                    nc.tensor.matmul(
                        ps.rearrange("p (a b) -> p a b", a=TPR),
                        lhsT=wfc_sb[:, k, vm * 128:(vm + 1) * 128],
                        rhs=hs_tile[:, :, k, :],
                        start=(k == 0),
                        stop=(k == KE - 1),
                    )
                lg = lgp.tile([128, 512], F32, tag="lg", name="lg")
                nc.scalar.activation(lg, ps, AF.Identity, bias=bfc_sb[:, vm:vm + 1])
                nc.sync.dma_start(
                    out=logT_d[vm * 128:(vm + 1) * 128, rn * 512:(rn + 1) * 512],
                    in_=lg)

            # XW for row chunk 0 must precede the loop
            for m in range(MG):
                emit_xw_chunk(0, m)

            # Per-step filler plan: phase rn covers transposes for chunk rn+1
            # (2 per step on steps 0-1), XW chunks rn+1 (3 on steps 2-7),
            # FC chunks rn-1 (2 per step from step 0).
            plan = {t: [] for t in range(T)}
            for rn in range(NRN):
                base = rn * TPR
                if rn + 1 < NRN:
                    for i, g in enumerate(range(4 * rn + 4, 4 * rn + 8)):
                        plan[base + i // 2].append(("tp", g))
                    for i in range(MG):
                        plan[base + 2 + i // 3].append(("xw", rn + 1, i))
                if rn > 0:
                    for vm in range(NVM):
                        plan[base + vm // 2].append(("fc", rn - 1, vm))

            # ---- recurrence with interleaved fillers ----
            cT_prev = None
            hs_cur = None
            hs_tiles = [None] * NRN
            for t in range(T):
                rn, toff = divmod(t, TPR)
                if toff == 0:
                    hs_cur = hsp.tile([128, TPR, KE, B], BF16, tag="hs", name="hs")
                    hs_tiles[rn] = hs_cur
                xw_t = xw_tiles[rn]
                cT_new = ctp.tile([128, KE, B], F32, tag="ct", name="ct")

                if t == 0:
                    if_sb = elemp.tile([128, 512], F32, tag="if", name="if_sb")
                    nc.scalar.activation(
                        if_sb.rearrange("p (m b) -> p m b", m=8),
                        xw_t[:, 0:8, 0:B], AF.Sigmoid)
                    g_sb = elemp.tile([128, 256], F32, tag="g", name="g_sb")
                    nc.scalar.activation(
                        g_sb.rearrange("p (m b) -> p m b", m=4),
                        xw_t[:, 8:12, 0:B], AF.Tanh)
                    o_sb = elemp.tile([128, 256], F32, tag="o", name="o_sb")
                    nc.scalar.activation(
                        o_sb.rearrange("p (m b) -> p m b", m=4),
                        xw_t[:, 12:16, 0:B], AF.Sigmoid)
                    nc.vector.tensor_mul(
                        cT_new.rearrange("p k b -> p (k b)"),
                        if_sb[:, 0:256], g_sb)
                else:
                    hprev_rn, hprev_off = divmod(t - 1, TPR)
                    hs_prev = hs_tiles[hprev_rn]
                    gp = gpsp.tile([128, 1024], F32, tag="gp", name="gp")
                    # preload xw into PSUM (sets has_written for accumulation)
                    nc.tensor.matmul(
                        gp[:, 0:512].rearrange("p (m b) -> p m b", m=8),
                        lhsT=ident, rhs=xw_t[:, 0:8, toff * B:(toff + 1) * B],
                        start=True, stop=False, skip_group_check=True)
                    nc.tensor.matmul(
                        gp[:, 512:1024].rearrange("p (m b) -> p m b", m=8),
                        lhsT=ident, rhs=xw_t[:, 8:16, toff * B:(toff + 1) * B],
                        start=True, stop=False, skip_group_check=True)
                    for m in M_ORDER:
                        for k in range(KE):
                            nc.tensor.matmul(
                                gp[:, m * B:(m + 1) * B],
                                lhsT=u_sb[:, k, m * 128:(m + 1) * 128],
                                rhs=hs_prev[:, hprev_off, k, :],
                                start=False, stop=(k == KE - 1),
                                skip_group_check=True)
                    # gate slices (i,f,c,o layout): c first for early tanh
                    g_sb = elemp.tile([128, 256], F32, tag="g", name="g_sb")
                    nc.scalar.activation(g_sb, gp[:, 512:768], AF.Tanh)
                    if_sb = elemp.tile([128, 512], F32, tag="if", name="if_sb")
                    nc.scalar.activation(if_sb[:, 256:512], gp[:, 256:512], AF.Sigmoid)
                    nc.scalar.activation(if_sb[:, 0:256], gp[:, 0:256], AF.Sigmoid)
                    o_sb = elemp.tile([128, 256], F32, tag="o", name="o_sb")
                    nc.scalar.activation(o_sb, gp[:, 768:1024], AF.Sigmoid)
                    fc_t = elemp.tile([128, 256], F32, tag="fc", name="fc_t")
                    nc.vector.tensor_mul(
                        fc_t, if_sb[:, 256:512],
                        cT_prev.rearrange("p k b -> p (k b)"))
                    ig_t = elemp.tile([128, 256], F32, tag="ig", name="ig_t")
                    nc.vector.tensor_mul(ig_t, if_sb[:, 0:256], g_sb)
                    nc.vector.tensor_add(
                        cT_new.rearrange("p k b -> p (k b)"), fc_t, ig_t)

                tanhc = elemp.tile([128, 256], F32, tag="tanhc", name="tanhc")
                nc.scalar.activation(
                    tanhc, cT_new.rearrange("p k b -> p (k b)"), AF.Tanh)
                # h_t = o * tanh(c) -> straight into hs^T (bf16)
                nc.vector.tensor_mul(
                    hs_cur[:, toff, :, :],
                    o_sb.rearrange("p (k b) -> p k b", k=KE),
                    tanhc.rearrange("p (k b) -> p k b", k=KE))
                cT_prev = cT_new

                for item in plan[t]:
                    if item[0] == "tp":
                        emit_transpose(item[1])
                    elif item[0] == "xw":
                        emit_xw_chunk(item[1], item[2])
                    else:
                        emit_fc_chunk(item[1], item[2], hs_tiles[item[1]])

            # drain remaining FC chunks (last row chunk)
            for vm in range(NVM):
                emit_fc_chunk(NRN - 1, vm, hs_tiles[NRN - 1])

    nc.compile()
    return nc


def _tile128(a, nchunk):
    """[nchunk*128, X] -> [128, nchunk*X] in (p, chunk, X) order."""
    n = a.shape[0] // 128
    assert n == nchunk
    return np.ascontiguousarray(
        a.reshape(n, 128, -1).transpose(1, 0, 2).reshape(128, -1))


def prep_inputs(inputs):
    """Host-side sharding/layout prep. Returns in_maps for 8 cores."""
    bf = lambda a: np.ascontiguousarray(np.asarray(a, np.float32)).astype(
        ml_dtypes.bfloat16)
    f32 = lambda a: np.ascontiguousarray(np.asarray(a, np.float32))

    features = f32(inputs["features"])
    captions = np.asarray(inputs["captions"]).astype(np.int32)
    wx = np.concatenate([f32(inputs["W_i"]), f32(inputs["W_f"]),
                         f32(inputs["W_c"]), f32(inputs["W_o"])], axis=1)
    u = np.concatenate([f32(inputs["U_i"]), f32(inputs["U_f"]),
                        f32(inputs["U_c"]), f32(inputs["U_o"])], axis=1)
    bc = np.concatenate([f32(inputs["b_i"]), f32(inputs["b_f"]),
                         f32(inputs["b_c"]), f32(inputs["b_o"])])
    wfc = f32(inputs["W_fc"])
    bfc = f32(inputs["b_fc"])

    # time-major token table for x rows 64..2047, packed [128, 16]
    tok = captions[:, :T - 1].T.reshape(-1)          # [(t-1)*64 + b], 1984
    tok = np.concatenate([tok, np.zeros(2048 - tok.size, np.int32)])
    tok2d = np.ascontiguousarray(tok.reshape(16, 128).T).astype(np.int32)

    common = {
        "featT": _tile128(bf(features.T), KIN),
        "wp": _tile128(bf(inputs["W_proj"]), KIN),
        "bp": f32(inputs["b_proj"]).reshape(KE, 128).T.copy(),
        "emb": bf(inputs["emb"]),
        "tok": tok2d,
        "wx": _tile128(bf(wx), KE),
        "u": _tile128(bf(u), KE),
        "bc": bc.reshape(MG, 128).T.copy(),
    }
    in_maps = []
    for c in range(N_CORES):
        wfc_k = np.zeros((H, VSP), np.float32)
        wfc_k[:, :VS] = wfc[:, c * VS:(c + 1) * VS]
        bfc_k = np.zeros((VSP,), np.float32)
        bfc_k[:VS] = bfc[c * VS:(c + 1) * VS]
        in_maps.append({
            **common,
            "wfc": _tile128(bf(wfc_k), KE),
            "bfc": bfc_k.reshape(NVM, 128).T.copy(),
        })
    return in_maps


def assemble(results):
    """results: list of 8 dicts with 'logT' [VSP, R] -> [B, T, V] f32."""
    full = np.concatenate([np.asarray(r["logT"])[:VS] for r in results], axis=0)
    return np.ascontiguousarray(
        full.reshape(V, T, B).transpose(2, 1, 0)).astype(np.float32)


_CACHED_NC = None


def _get_nc():
    global _CACHED_NC
    if _CACHED_NC is None:
        _CACHED_NC = build_program()
        _CACHED_NC.m = get_hw_module(_CACHED_NC.m)
    return _CACHED_NC


def run_on_hw(inputs, trace=False):
    nc = _get_nc()
    in_maps = prep_inputs(inputs)
    res = bass_utils.run_bass_kernel_spmd(
        nc, in_maps, core_ids=list(range(N_CORES)), trace=trace)
    return assemble(res.results), res.exec_time_ns


def kernel(**inputs):
    out, _ = run_on_hw(inputs, trace=False)
    return out


# revision 9
# speedup vs baseline: 1.7493x; 1.1340x over previous
"""Trainium2 Bass kernel for nn_DecoderRNN (LSTM image-caption decoder).

Model: feat = features @ W_proj + b_proj;  x = [feat, emb[captions[:, :-1]]]
       LSTM over T=32 steps (batch 64, hidden 512), logits = hs @ W_fc + b_fc.

Distribution across 8 NeuronCores (no collectives):
  - The sequential LSTM recurrence is latency-bound (M=64), so it is
    REPLICATED on every core; each core therefore holds the full hidden
    trajectory locally.
  - The dominant FC layer (512x10000) and the 82MB output are SHARDED by
    vocab: core k computes logits[:, :, k*1250:(k+1)*1250].

Per-core kernel layout (everything transposed: partition = feature dim,
free = (time, batch)), all matmuls bf16 with fp32 PSUM accumulation:
  - weights arrive host-pretiled so every load is one contiguous DMA
  - proj: featT_E[m] = W_proj[:,m]^T @ features^T
  - embedding rows gathered by indirect DMA, transposed on the PE
    (128x128 transpose + copy) into X^T row-chunk tiles
  - XW^T = Wx^T @ X^T + b, one row chunk (8 steps) at a time
  - recurrence per step: 2 identity-matmuls preload xw into PSUM (sets
    has_written), 64 U-matmuls accumulate, gate order c,f,i,o so the
    ACT/DVE chain starts early; h_t written bf16 straight into hs^T
  - FC logitsT = W_fc^T @ hs^T + b_fc per row chunk, streamed to DRAM
  Transposes, XW chunks of row-chunk rn+1 and FC chunks of rn-1 are
  emitted BETWEEN recurrence steps: each engine executes its queue in
  order, so this fills the tensor engine's dependency-chain gaps (HAM
  stays warm) without delaying the next step.
"""

import numpy as np
import ml_dtypes

import concourse.bass as bass
import concourse.mybir as mybir
import concourse.tile as tile
from concourse import bacc
from concourse import bass_utils
from concourse.bass_interp import get_hw_module
from concourse.masks import make_identity

BF16 = mybir.dt.bfloat16
F32 = mybir.dt.float32
I32 = mybir.dt.int32
AF = mybir.ActivationFunctionType
ALU = mybir.AluOpType

B, T, E, H, V, IN = 64, 32, 512, 512, 10000, 2048
N_CORES = 8
VS = V // N_CORES          # 1250 vocab rows per core
VSP = 1280                 # padded to 10 x 128
G4 = 4 * H                 # 2048 gate columns, order (i, f, c, o)
R = B * T                  # 2048 rows, time-major: r = t*64 + b
KE = E // 128              # 4 contraction chunks over E/H
KIN = IN // 128            # 16 chunks over INPUT
MG = G4 // 128             # 16 gate chunks
NRN = 4                    # row chunks of 512 (8 timesteps each)
TPR = T // NRN             # 8 timesteps per row chunk
NVM = VSP // 128           # 10 vocab chunks per core

# xw is stored in slot order (c, f, i, o); PERM maps original gate-chunk
# index (i,f,c,o layout) to its storage slot.
PERM = [8, 9, 10, 11, 4, 5, 6, 7, 0, 1, 2, 3, 12, 13, 14, 15]
# U-matmul emission: first the (c, f) psum tile, then (i, o)
M_CF = [8, 9, 10, 11, 4, 5, 6, 7]
M_IO = [0, 1, 2, 3, 12, 13, 14, 15]


def _row_segments(g):
    """Token tile g holds x rows r0..r0+w; split by row-chunk boundary."""
    r0 = B + g * 128
    w = min(128, R - r0)
    segs = []
    j = 0
    while j < w:
        rn, off = divmod(r0 + j, 512)
        seg = min(w - j, 512 - off)
        segs.append((j, rn, off, seg))
        j += seg
    return segs


def build_program():
    nc = bacc.Bacc("TRN2", target_bir_lowering=False, debug=False)

    featT_d = nc.dram_tensor("featT", [128, KIN * B], BF16, kind="ExternalInput").ap()
    wp_d = nc.dram_tensor("wp", [128, KIN * E], BF16, kind="ExternalInput").ap()
    bp_d = nc.dram_tensor("bp", [128, KE], F32, kind="ExternalInput").ap()
    emb_d = nc.dram_tensor("emb", [V, E], BF16, kind="ExternalInput").ap()
    tok_d = nc.dram_tensor("tok", [128, 16], I32, kind="ExternalInput").ap()
    wx_d = nc.dram_tensor("wx", [128, KE * G4], BF16, kind="ExternalInput").ap()
    u_d = nc.dram_tensor("u", [128, KE * G4], BF16, kind="ExternalInput").ap()
    bc_d = nc.dram_tensor("bc", [128, MG], F32, kind="ExternalInput").ap()
    wfc_d = nc.dram_tensor("wfc", [128, KE * VSP], BF16, kind="ExternalInput").ap()
    bfc_d = nc.dram_tensor("bfc", [128, NVM], F32, kind="ExternalInput").ap()
    logT_d = nc.dram_tensor("logT", [VSP, R], F32, kind="ExternalOutput").ap()

    with tile.TileContext(nc) as tc:
        with (
            tc.tile_pool(name="const", bufs=1) as const,
            tc.tile_pool(name="xt", bufs=1) as xtp,
            tc.tile_pool(name="xw", bufs=2) as xwp,
            tc.tile_pool(name="gath", bufs=4) as gathp,
            tc.tile_pool(name="hs", bufs=2) as hsp,
            tc.tile_pool(name="ct", bufs=2) as ctp,
            tc.tile_pool(name="elem", bufs=2) as elemp,
            tc.tile_pool(name="lg", bufs=3) as lgp,
            tc.tile_pool(name="gps", bufs=2, space="PSUM") as gpsp,
            tc.tile_pool(name="mmps", bufs=2, space="PSUM") as mmpsp,
            tc.tile_pool(name="tps", bufs=2, space="PSUM") as tpsp,
        ):
            # ---- weights/constants to SBUF (contiguous, need-ordered) ----
            tok_sb = const.tile([128, 16], I32)
            nc.sync.dma_start(out=tok_sb, in_=tok_d)
            wp_sb = const.tile([128, KIN, E], BF16)
            nc.sync.dma_start(out=wp_sb.rearrange("p k e -> p (k e)"), in_=wp_d)
            featT_sb = const.tile([128, KIN, B], BF16)
            nc.sync.dma_start(out=featT_sb.rearrange("p k b -> p (k b)"), in_=featT_d)
            wx_sb = const.tile([128, KE, G4], BF16)
            nc.sync.dma_start(out=wx_sb.rearrange("p k j -> p (k j)"), in_=wx_d)
            u_sb = const.tile([128, KE, G4], BF16)
            nc.sync.dma_start(out=u_sb.rearrange("p k j -> p (k j)"), in_=u_d)
            wfc_sb = const.tile([128, KE, VSP], BF16)
            nc.sync.dma_start(out=wfc_sb.rearrange("p k v -> p (k v)"), in_=wfc_d)
            bp_sb = const.tile([128, KE], F32)
            nc.sync.dma_start(out=bp_sb, in_=bp_d)
            bc_sb = const.tile([128, MG], F32)
            nc.sync.dma_start(out=bc_sb, in_=bc_d)
            bfc_sb = const.tile([128, NVM], F32)
            nc.sync.dma_start(out=bfc_sb, in_=bfc_d)
            ident = const.tile([128, 128], BF16)
            make_identity(nc, ident)

            # X^T row-chunk tiles: [E-within-chunk, E-chunk, row-in-chunk]
            xT = [xtp.tile([128, KE, 512], BF16, tag=f"xt{rn}", name=f"xt{rn}")
                  for rn in range(NRN)]

            # ---- all embedding gathers up front (gpsimd queue) ----
            gts = []
            for g in range(16):
                gt = gathp.tile([128, E], BF16, tag="gath", name="gt")
                nc.gpsimd.indirect_dma_start(
                    out=gt,
                    out_offset=None,
                    in_=emb_d,
                    in_offset=bass.IndirectOffsetOnAxis(ap=tok_sb[:, g:g + 1], axis=0),
                )
                gts.append(gt)

            def emit_transpose(g):
                """PE-transpose token tile g into its X^T slots."""
                gt = gts[g]
                segs = _row_segments(g)
                for e in range(KE):
                    tp = tpsp.tile([128, 128], BF16, tag="tp", name="tp")
                    nc.tensor.transpose(tp, gt[:, e * 128:(e + 1) * 128], ident)
                    for (j, rn, off, seg) in segs:
                        nc.scalar.activation(
                            xT[rn][:, e, off:off + seg], tp[:, j:j + seg], AF.Copy)

            # ---- feature projection -> x^T rows 0..63 (chunk 0) ----
            for m in range(KE):
                ps = mmpsp.tile([128, 512], F32, tag="mmps", name="ps")
                for k in range(KIN):
                    nc.tensor.matmul(
                        ps[:, :B],
                        lhsT=wp_sb[:, k, m * 128:(m + 1) * 128],
                        rhs=featT_sb[:, k, :],
                        start=(k == 0),
                        stop=(k == KIN - 1),
                    )
                nc.scalar.activation(
                    xT[0][:, m, 0:B], ps[:, :B], AF.Identity,
                    bias=bp_sb[:, m:m + 1],
                )

            for g in range(4):
                emit_transpose(g)

            xw_tiles = [None] * NRN

            def emit_xw_chunk(rn, m):
                if xw_tiles[rn] is None:
                    xw_tiles[rn] = xwp.tile([128, MG, 512], BF16, tag="xw",
                                            name=f"xw{rn}")
                xw_t = xw_tiles[rn]
                ps = mmpsp.tile([128, 512], F32, tag="mmps", name="ps")
                for k in range(KE):
                    nc.tensor.matmul(
                        ps,
                        lhsT=wx_sb[:, k, m * 128:(m + 1) * 128],
                        rhs=xT[rn][:, k, :],
                        start=(k == 0),
                        stop=(k == KE - 1),
                    )
                # copy + gate bias on DVE (ACT is the busier engine in-loop)
                nc.vector.tensor_scalar(
                    out=xw_t[:, PERM[m], :], in0=ps, scalar1=bc_sb[:, m:m + 1],
                    scalar2=None, op0=ALU.add)

            def emit_fc_chunk(rn, vm, hs_tile):
                ps = mmpsp.tile([128, 512], F32, tag="mmps", name="ps")
                for k in range(KE):
                    nc.tensor.matmul(
                        ps.rearrange("p (a b) -> p a b", a=TPR),
                        lhsT=wfc_sb[:, k, vm * 128:(vm + 1) * 128],
                        rhs=hs_tile[:, :, k, :],
                        start=(k == 0),
                        stop=(k == KE - 1),
                    )
                lg = lgp.tile([128, 512], F32, tag="lg", name="lg")
                nc.scalar.activation(lg[:, 0:256], ps[:, 0:256], AF.Identity,
                                     bias=bfc_sb[:, vm:vm + 1])
                nc.scalar.activation(lg[:, 256:512], ps[:, 256:512], AF.Identity,
                                     bias=bfc_sb[:, vm:vm + 1])
                nc.sync.dma_start(
                    out=logT_d[vm * 128:(vm + 1) * 128, rn * 512:(rn + 1) * 512],
                    in_=lg)

            # XW for row chunk 0 must precede the loop
            for m in range(MG):
                emit_xw_chunk(0, m)

            # Per-step filler plan: phase rn covers transposes for chunk rn+1
            # (2 per step on steps 0-1), XW chunks rn+1 (3 on steps 2-7),
            # FC chunks rn-1 (2 per step from step 0).
            plan = {t: [] for t in range(T)}
            for rn in range(NRN):
                base = rn * TPR
                if rn + 1 < NRN:
                    for i, g in enumerate(range(4 * rn + 4, 4 * rn + 8)):
                        plan[base + i // 2].append(("tp", g))
                    for i in range(MG):
                        plan[base + 2 + i // 3].append(("xw", rn + 1, i))
                if rn > 0:
                    for vm in range(NVM):
                        plan[base + vm // 2].append(("fc", rn - 1, vm))

            # ---- recurrence with interleaved fillers ----
            cT_prev = None
            hs_cur = None
            hs_tiles = [None] * NRN
            for t in range(T):
                rn, toff = divmod(t, TPR)
                if toff == 0:
                    hs_cur = hsp.tile([128, TPR, KE, B], BF16, tag="hs", name="hs")
                    hs_tiles[rn] = hs_cur
                xw_t = xw_tiles[rn]
                cT_new = ctp.tile([128, KE, B], F32, tag="ct", name="ct")

                if t == 0:
                    g_sb = elemp.tile([128, 256], F32, tag="g", name="g_sb")
                    nc.scalar.activation(
                        g_sb.rearrange("p (m b) -> p m b", m=4),
                        xw_t[:, 0:4, 0:B], AF.Tanh)
                    if_sb = elemp.tile([128, 512], F32, tag="if", name="if_sb")
                    nc.scalar.activation(
                        if_sb[:, 256:512].rearrange("p (m b) -> p m b", m=4),
                        xw_t[:, 4:8, 0:B], AF.Sigmoid)
                    nc.scalar.activation(
                        if_sb[:, 0:256].rearrange("p (m b) -> p m b", m=4),
                        xw_t[:, 8:12, 0:B], AF.Sigmoid)
                    o_sb = elemp.tile([128, 256], F32, tag="o", name="o_sb")
                    nc.scalar.activation(
                        o_sb.rearrange("p (m b) -> p m b", m=4),
                        xw_t[:, 12:16, 0:B], AF.Sigmoid)
                    nc.vector.tensor_mul(
                        cT_new.rearrange("p k b -> p (k b)"),
                        if_sb[:, 0:256], g_sb)
                else:
                    hprev_rn, hprev_off = divmod(t - 1, TPR)
                    hs_prev = hs_tiles[hprev_rn]
                    gp_cf = gpsp.tile([128, 512], F32, tag="gpcf", name="gp_cf")
                    gp_io = gpsp.tile([128, 512], F32, tag="gpio", name="gp_io")
                    # preload xw into PSUM (sets has_written for accumulation)
                    nc.tensor.matmul(
                        gp_cf.rearrange("p (m b) -> p m b", m=8),
                        lhsT=ident, rhs=xw_t[:, 0:8, toff * B:(toff + 1) * B],
                        start=True, stop=False, skip_group_check=True)
                    for m in M_CF:
                        sl = PERM[m] * B      # slot within (c,f,i,o) layout
                        for k in range(KE):
                            nc.tensor.matmul(
                                gp_cf[:, sl:sl + B],
                                lhsT=u_sb[:, k, m * 128:(m + 1) * 128],
                                rhs=hs_prev[:, hprev_off, k, :],
                                start=False, stop=(k == KE - 1),
                                skip_group_check=True)
                    nc.tensor.matmul(
                        gp_io.rearrange("p (m b) -> p m b", m=8),
                        lhsT=ident, rhs=xw_t[:, 8:16, toff * B:(toff + 1) * B],
                        start=True, stop=False, skip_group_check=True)
                    for m in M_IO:
                        sl = (PERM[m] - 8) * B
                        for k in range(KE):
                            nc.tensor.matmul(
                                gp_io[:, sl:sl + B],
                                lhsT=u_sb[:, k, m * 128:(m + 1) * 128],
                                rhs=hs_prev[:, hprev_off, k, :],
                                start=False, stop=(k == KE - 1),
                                skip_group_check=True)
                    # chain starts once the (c,f) tile is done
                    g_sb = elemp.tile([128, 256], F32, tag="g", name="g_sb")
                    nc.scalar.activation(g_sb, gp_cf[:, 0:256], AF.Tanh)
                    if_sb = elemp.tile([128, 512], F32, tag="if", name="if_sb")
                    nc.scalar.activation(if_sb[:, 256:512], gp_cf[:, 256:512], AF.Sigmoid)
                    nc.scalar.activation(if_sb[:, 0:256], gp_io[:, 0:256], AF.Sigmoid)
                    o_sb = elemp.tile([128, 256], F32, tag="o", name="o_sb")
                    nc.scalar.activation(o_sb, gp_io[:, 256:512], AF.Sigmoid)
                    fc_t = elemp.tile([128, 256], F32, tag="fc", name="fc_t")
                    nc.vector.tensor_mul(
                        fc_t, if_sb[:, 256:512],
                        cT_prev.rearrange("p k b -> p (k b)"))
                    # i*g on the (otherwise idle) Pool engine, parallel to DVE
                    ig_t = elemp.tile([128, 256], F32, tag="ig", name="ig_t")
                    nc.gpsimd.tensor_mul(ig_t, if_sb[:, 0:256], g_sb)
                    nc.vector.tensor_add(
                        cT_new.rearrange("p k b -> p (k b)"), fc_t, ig_t)

                tanhc = elemp.tile([128, 256], F32, tag="tanhc", name="tanhc")
                nc.scalar.activation(
                    tanhc, cT_new.rearrange("p k b -> p (k b)"), AF.Tanh)
                # h_t = o * tanh(c) -> straight into hs^T (bf16)
                nc.vector.tensor_mul(
                    hs_cur[:, toff, :, :],
                    o_sb.rearrange("p (k b) -> p k b", k=KE),
                    tanhc.rearrange("p (k b) -> p k b", k=KE))
                cT_prev = cT_new

                for item in plan[t]:
                    if item[0] == "tp":
                        emit_transpose(item[1])
                    elif item[0] == "xw":
                        emit_xw_chunk(item[1], item[2])
                    else:
                        emit_fc_chunk(item[1], item[2], hs_tiles[item[1]])

            # drain remaining FC chunks (last row chunk)
            for vm in range(NVM):
                emit_fc_chunk(NRN - 1, vm, hs_tiles[NRN - 1])

    nc.compile()
    return nc


def _tile128(a, nchunk):
    """[nchunk*128, X] -> [128, nchunk*X] in (p, chunk, X) order."""
    n = a.shape[0] // 128
    assert n == nchunk
    return np.ascontiguousarray(
        a.reshape(n, 128, -1).transpose(1, 0, 2).reshape(128, -1))


def prep_inputs(inputs):
    """Host-side sharding/layout prep. Returns in_maps for 8 cores."""
    bf = lambda a: np.ascontiguousarray(np.asarray(a, np.float32)).astype(
        ml_dtypes.bfloat16)
    f32 = lambda a: np.ascontiguousarray(np.asarray(a, np.float32))

    features = f32(inputs["features"])
    captions = np.asarray(inputs["captions"]).astype(np.int32)
    wx = np.concatenate([f32(inputs["W_i"]), f32(inputs["W_f"]),
                         f32(inputs["W_c"]), f32(inputs["W_o"])], axis=1)
    u = np.concatenate([f32(inputs["U_i"]), f32(inputs["U_f"]),
                        f32(inputs["U_c"]), f32(inputs["U_o"])], axis=1)
    bc = np.concatenate([f32(inputs["b_i"]), f32(inputs["b_f"]),
                         f32(inputs["b_c"]), f32(inputs["b_o"])])
    wfc = f32(inputs["W_fc"])
    bfc = f32(inputs["b_fc"])

    # time-major token table for x rows 64..2047, packed [128, 16]
    tok = captions[:, :T - 1].T.reshape(-1)          # [(t-1)*64 + b], 1984
    tok = np.concatenate([tok, np.zeros(2048 - tok.size, np.int32)])
    tok2d = np.ascontiguousarray(tok.reshape(16, 128).T).astype(np.int32)

    common = {
        "featT": _tile128(bf(features.T), KIN),
        "wp": _tile128(bf(inputs["W_proj"]), KIN),
        "bp": f32(inputs["b_proj"]).reshape(KE, 128).T.copy(),
        "emb": bf(inputs["emb"]),
        "tok": tok2d,
        "wx": _tile128(bf(wx), KE),
        "u": _tile128(bf(u), KE),
        "bc": bc.reshape(MG, 128).T.copy(),
    }
    in_maps = []
    for c in range(N_CORES):
        wfc_k = np.zeros((H, VSP), np.float32)
        wfc_k[:, :VS] = wfc[:, c * VS:(c + 1) * VS]
        bfc_k = np.zeros((VSP,), np.float32)
        bfc_k[:VS] = bfc[c * VS:(c + 1) * VS]
        in_maps.append({
            **common,
            "wfc": _tile128(bf(wfc_k), KE),
            "bfc": bfc_k.reshape(NVM, 128).T.copy(),
        })
    return in_maps


def assemble(results):
    """results: list of 8 dicts with 'logT' [VSP, R] -> [B, T, V] f32."""
    full = np.concatenate([np.asarray(r["logT"])[:VS] for r in results], axis=0)
    return np.ascontiguousarray(
        full.reshape(V, T, B).transpose(2, 1, 0)).astype(np.float32)


_CACHED_NC = None


def _get_nc():
    global _CACHED_NC
    if _CACHED_NC is None:
        _CACHED_NC = build_program()
        _CACHED_NC.m = get_hw_module(_CACHED_NC.m)
    return _CACHED_NC


def run_on_hw(inputs, trace=False):
    nc = _get_nc()
    in_maps = prep_inputs(inputs)
    res = bass_utils.run_bass_kernel_spmd(
        nc, in_maps, core_ids=list(range(N_CORES)), trace=trace)
    return assemble(res.results), res.exec_time_ns


def kernel(**inputs):
    out, _ = run_on_hw(inputs, trace=False)
    return out
